# revision 43
# baseline (speedup 1.0000x reference)
"""Multi-head attention (dense transformer block) on 8 Trainium2 NeuronCores.

Sharding: (batch=4) x (head-group=2) -> 8 shards, tensor-parallel over heads.
Core c handles batch b = c//2 and heads [8*hg, 8*hg+8) with hg = c%2:
Q/K/V weights column-sharded (512 of 1024 output dims per core), Wo
row-sharded; the two row-parallel partial outputs per batch element are
summed host-side (plus the bo bias) during the unshard. No collectives.

Inputs are host-packed so every DMA moves >=8KB-contiguous per-partition
lines (descriptor-dispatch efficiency), issued in first-use order so the
first projection matmul only waits on ~2MB, not the whole 9MB input set.

Per core: V projection into a per-head 65-column interleave (trailing ones
column fuses the softmax denominator into the attn.V matmul); per head pair
p (0..3): Q/K projected transposed [128, 2048]; per 512-query block, scores
for both heads land in one 2-bank PSUM tile via a row-paired matmul pair,
one wide exp(St/8) on ACT, and one M=65 matmul per head accumulates
[V|ones].T @ Pt over the 16 key chunks.  Numerators are evacuated
unnormalized (bf16) on DVE, the Z-row reciprocal runs eagerly on DVE, and
the 1/Z broadcast (ones outer-product matmuls) + in-place normalization is
deferred one query block so the PE never stalls on it.  The O projection
is interleaved into pair 3's attention (one 512-token group per query
block), so the output drains while pair 3's ACT-bound exp stream runs.
Output is stored bf16; the host upcasts, sums the two row-parallel
partials and adds bo.

Matmul operands are bf16 (fp32 PSUM accumulation).
"""

import numpy as np
import ml_dtypes

import concourse.bass as bass
import concourse.tile as tile
import concourse.mybir as mybir
from concourse.bass_utils import run_bass_kernel_spmd

F32 = mybir.dt.float32
F32R = mybir.dt.float32r
BF16 = mybir.dt.bfloat16
EXP = mybir.ActivationFunctionType.Exp

D = 1024          # d_model
S = 2048          # sequence length (full batch element per core)
NH = 16           # heads total
NHC = 8           # heads per core
NP = 4            # head pairs per core
DH = 64           # head dim
DC = 512          # output dims per core (NHC * DH)
NCORES = 8
VARIANT = 7740     # bump to bust the HLO-signature-keyed NEFF cache


def split_multi_waits(nc):
    """The walrus build in this container accepts at most one sync-wait per
    instruction; move extra waits onto same-engine nops inserted before the
    offending instruction."""
    k = 0
    for f in nc.m.functions:
        for bb in f.blocks:
            out, changed = [], False
            for inst in bb.instructions:
                si = inst.sync_info
                waits = list(si.on_wait) if si and si.on_wait else []
                if len(waits) > 1:
                    changed = True
                    for w in waits[:-1]:
                        nop = mybir.InstNoOp(name=f"wsplit-{k}", ins=[], outs=[])
                        k += 1
                        nop.engine = inst.engine
                        nop.sync_info = mybir.SyncInfo(on_wait=[w], on_update=[])
                        nc.register_instruction(nop, overwrite=True)
                        out.append(nop)
                    si.on_wait = waits[-1:]
                out.append(inst)
            if changed:
                bb.instructions = out


def build_program():
    nc = bass.Bass()
    # packed: [p, 512*d + c] = W?T[128*d + p, c]
    wkP = nc.declare_dram_parameter("wkP", [128, 4096], BF16, isOutput=False)
    wqP = nc.declare_dram_parameter("wqP", [128, 4096], BF16, isOutput=False)
    wvP = nc.declare_dram_parameter("wvP", [128, 4096], BF16, isOutput=False)
    # packed: [p, 4096*tb + 512*d + c] = x[b][512*tb + c, 128*d + p]
    xP = nc.declare_dram_parameter("xP", [128, 16384], BF16, isOutput=False)
    # packed: [p, 1024*d + c] = WoT_shard[128*d + p, c]
    woP = nc.declare_dram_parameter("woP", [128, 4096], BF16, isOutput=False)
    bk2 = nc.declare_dram_parameter("bk2", [128, NP], F32, isOutput=False)
    bq2 = nc.declare_dram_parameter("bq2", [128, NP], F32, isOutput=False)
    bvb = nc.declare_dram_parameter("bvb", [128, DC], F32, isOutput=False)
    ones4 = nc.declare_dram_parameter("ones4", [4, 64], F32R, isOutput=False)
    tag = nc.declare_dram_parameter("tag", [1, VARIANT], F32, isOutput=False)
    out = nc.declare_dram_parameter("out", [S, D], BF16, isOutput=True)

    with tile.TileContext(nc) as tc:
        with tc.tile_pool(name="pp", bufs=1) as pp, \
             tc.tile_pool(name="qk", bufs=2) as qkp, \
             tc.tile_pool(name="pt", bufs=3) as ptp, \
             tc.tile_pool(name="zr", bufs=2) as zrp, \
             tc.tile_pool(name="osb", bufs=3) as op_, \
             tc.tile_pool(name="psS", bufs=2, space="PSUM") as stp, \
             tc.tile_pool(name="psA", bufs=4, space="PSUM") as psa:
            bk_sb = pp.tile([128, 16], F32, name="bk_sb", tag="bk_sb")
            bq_sb = pp.tile([128, 16], F32, name="bq_sb", tag="bq_sb")
            bvb_sb = pp.tile([128, DC], F32, name="bvb_sb", tag="bvb_sb")
            ones_sb = pp.tile([128, 64], F32R, name="ones_sb", tag="ones_sb")
            tag_sb = pp.tile([1, VARIANT], F32, name="tag_sb", tag="tag_sb")
            wk_sb = pp.tile([128, 4096], BF16, name="wk_sb", tag="wk_sb")
            wq_sb = pp.tile([128, 4096], BF16, name="wq_sb", tag="wq_sb")
            wv_sb = pp.tile([128, 4096], BF16, name="wv_sb", tag="wv_sb")
            wo_sb = pp.tile([128, 4096], BF16, name="wo_sb", tag="wo_sb")
            xt_sb = pp.tile([128, 16384], BF16, name="xt_sb", tag="xt_sb")

            def wk(d, p4=None):
                a = wk_sb[:, 512 * d:512 * (d + 1)]
                return a if p4 is None else a[:, 128 * p4:128 * (p4 + 1)]

            def wq(d, p4=None):
                a = wq_sb[:, 512 * d:512 * (d + 1)]
                return a if p4 is None else a[:, 128 * p4:128 * (p4 + 1)]

            def wv(d):
                return wv_sb[:, 512 * d:512 * (d + 1)]

            def wo(d, hf):
                return wo_sb[:, 1024 * d + 512 * hf:1024 * d + 512 * (hf + 1)]

            def xt512(d, tb):
                o = 4096 * tb + 512 * d
                return xt_sb[:, o:o + 512]

            def xt128(d, k):
                o = 4096 * (k // 4) + 512 * d + 128 * (k % 4)
                return xt_sb[:, o:o + 128]

            # DMAs in first-use order, split across the two HWDGE queues
            # (SP + Act) so the first projection's inputs land in parallel.
            nc.sync.dma_start(wk_sb[:, 0:2048], wkP[:, 0:2048])
            nc.scalar.dma_start(wk_sb[:, 2048:4096], wkP[:, 2048:4096])
            nc.scalar.dma_start(bk_sb[:, 0:NP], bk2[:])
            nc.scalar.dma_start(bq_sb[:, 0:NP], bq2[:])
            nc.scalar.dma_start(tag_sb[:], tag[:])
            for s in range(4):
                nc.scalar.dma_start(ones_sb[32 * s:32 * s + 1, :],
                                    ones4[s:s + 1, :])
            nc.sync.dma_start(xt_sb[:, 0:2048], xP[:, 0:2048])
            nc.scalar.dma_start(xt_sb[:, 2048:4096], xP[:, 2048:4096])
            nc.sync.dma_start(wq_sb[:], wqP[:])
            nc.scalar.dma_start(bvb_sb[:], bvb[:])
            nc.scalar.dma_start(wv_sb[:], wvP[:])
            for tb in range(1, 4):
                nc.sync.dma_start(xt_sb[:, 4096 * tb:4096 * (tb + 1)],
                                  xP[:, 4096 * tb:4096 * (tb + 1)])
            nc.scalar.dma_start(wo_sb[:], woP[:])

            vg = [pp.tile([128, 520], BF16, name=f"vg{t}", tag=f"vg{t}")
                  for t in range(16)]
            ots = [pp.tile([128, S], BF16, name=f"ot{i}", tag=f"ot{i}")
                   for i in range(NP)]

            def qk_tiles():
                kt_p = qkp.tile([128, S], BF16, name="kt_p", tag="kt", bufs=2)
                qt_p = qkp.tile([128, S], BF16, name="qt_p", tag="qt", bufs=2)
                return kt_p, qt_p

            def qk_group(p, tb, kt_p, qt_p):
                """K+Q projection for one 512-token block of head pair p."""
                ts = slice(512 * tb, 512 * (tb + 1))
                ps = psa.tile([128, 512], F32, name="psk", tag="psa", bufs=4)
                for d in range(8):
                    nc.tensor.matmul(ps[:], wk(d, p), xt512(d, tb),
                                     start=(d == 0), stop=(d == 7))
                nc.vector.tensor_scalar_add(kt_p[:, ts], ps[:],
                                            bk_sb[:, p:p + 1])
                ps = psa.tile([128, 512], F32, name="psq", tag="psa", bufs=4)
                for d in range(8):
                    nc.tensor.matmul(ps[:], wq(d, p), xt512(d, tb),
                                     start=(d == 0), stop=(d == 7))
                nc.vector.tensor_scalar_add(qt_p[:, ts], ps[:],
                                            bq_sb[:, p:p + 1])

            def make_tail(p, qs2, rz):
                """1/Z broadcast + normalization for one finished query
                block.  Emitted one block LATE so the PE's in-order stream
                never stalls (the DVE reciprocal ran a full block earlier).
                The broadcast lands in the st pool whose slots are freed
                promptly by ACT's exp, keeping the psa ring a clean 4-deep
                rotation of po0/po1/psk/psq."""
                def tail():
                    pb = stp.tile([128, 1024], F32, name="pb", tag="st",
                                  bufs=2)
                    nc.tensor.matmul(pb[0:64, 0:512], ones_sb[0:1, :],
                                     rz[0:1, :], start=True, stop=True)
                    nc.tensor.matmul(pb[0:64, 512:1024], ones_sb[32:33, :],
                                     rz[32:33, :], start=True, stop=True)
                    nc.vector.tensor_mul(ots[p][0:64, qs2], ots[p][0:64, qs2],
                                         pb[0:64, 0:512])
                    nc.vector.tensor_mul(ots[p][64:128, qs2],
                                         ots[p][64:128, qs2],
                                         pb[0:64, 512:1024])
                return tail

            def oproj_t8(t8):
                """O projection + store for one 128-token chunk."""
                osb = op_.tile([128, 1024], BF16, name="osb", tag="osb",
                               bufs=3)
                for hf in range(2):
                    ps = psa.tile([128, 512], F32, name="pso", tag="psa",
                                  bufs=4)
                    for p in range(4):
                        nc.tensor.matmul(
                            ps[:], ots[p][:, 128 * t8:128 * (t8 + 1)],
                            wo(p, hf), start=(p == 0), stop=(p == 3))
                    nc.vector.tensor_copy(osb[:, 512 * hf:512 * (hf + 1)],
                                          ps[:])
                nc.sync.dma_start(out[128 * t8:128 * (t8 + 1), :], osb[:])

            def oproj_group(g):
                """O projection + store for tokens [512g, 512(g+1))."""
                for t8 in range(4 * g, 4 * g + 4):
                    oproj_t8(t8)

            pending = None
            cur = qk_tiles()
            for tb in range(4):
                qk_group(0, tb, *cur)
            for p in range(NP):
                kt_p, qt_p = cur
                # next pair's projection groups are woven in one per query
                # block below, filling the PE bubble at each block boundary
                nxt = qk_tiles() if p + 1 < NP else None
                c0, c1 = 130 * p, 130 * p + 65
                for qb in range(4):
                    qs2 = slice(512 * qb, 512 * (qb + 1))
                    po0 = psa.tile([128, 512], F32, name="po0", tag="psa", bufs=4)
                    po1 = psa.tile([128, 512], F32, name="po1", tag="psa", bufs=4)
                    for k in range(16):
                        if k == 8 and pending is not None:
                            # previous block's 1/Z broadcast + normalization,
                            # woven mid-loop: DVE is idle here, so the mults
                            # finish long before anything reuses pb's slot
                            # or (pair 3) the O projection reads ots
                            pending()
                            pending = None
                        if p == NP - 1 and qb >= 1 and 10 <= k <= 13:
                            # pair 3's k-loop is ACT-paced (exp 1081ns vs PE
                            # 861ns per iter): weave the previous block's O
                            # projection chains in so PE stays busy instead
                            # of micro-stalling on exp, and the block ends
                            # as soon as the accumulators close
                            oproj_t8(4 * (qb - 1) + (k - 10))
                        if p == 0 and qb == 0:
                            # fused V projection for key chunk k
                            ps = psa.tile([128, 512], F32, name="psv",
                                          tag="psa", bufs=4)
                            for d in range(8):
                                nc.tensor.matmul(
                                    ps[:], xt128(d, k), wv(d),
                                    start=(d == 0), stop=(d == 7))
                            dst = vg[k][:].rearrange(
                                "p (h w) -> p h w", w=65)[:, :, 0:64]
                            nc.vector.tensor_add(
                                dst, ps[:].rearrange("p (h w) -> p h w", w=64),
                                bvb_sb[:].rearrange("p (h w) -> p h w", w=64))
                            nc.vector.memset(
                                vg[k][:].rearrange(
                                    "p (h w) -> p h w", w=65)[:, :, 64:65], 1.0)
                        ks = slice(128 * k, 128 * (k + 1))
                        st = stp.tile([128, 1024], F32, name="st", tag="st",
                                      bufs=2)
                        nc.tensor.matmul(st[:, 0:512], kt_p[0:64, ks],
                                         qt_p[0:64, qs2], start=True, stop=True)
                        nc.tensor.matmul(st[:, 512:1024], kt_p[64:128, ks],
                                         qt_p[64:128, qs2], start=True, stop=True)
                        pt = ptp.tile([128, 1024], BF16, name="pt", tag="pt",
                                      bufs=3)
                        nc.scalar.activation(pt[:], st[:], EXP, scale=0.125)
                        first, last = (k == 0), (k == 15)
                        nc.tensor.matmul(po0[0:65, :], vg[k][:, c0:c0 + 65],
                                         pt[:, 0:512], start=first, stop=last)
                        nc.tensor.matmul(po1[0:65, :], vg[k][:, c1:c1 + 65],
                                         pt[:, 512:1024], start=first, stop=last)
                    # numerator evacuation leads the DVE queue so the po
                    # slots recycle before the next block's first attnV;
                    # the Z-row copies + reciprocal follow (their consumer,
                    # the 1/Z broadcast, fires at k==8 of the next block).
                    # The very last block inverts this: nothing reuses its
                    # po slots, and the drain chain hangs on the reciprocal.
                    last = (p == NP - 1 and qb == 3)
                    zr = zrp.tile([128, 512], F32, name="zrt", tag="zrt", bufs=2)
                    rz = zrp.tile([128, 512], F32R, name="rz", tag="rz", bufs=2)
                    if not last:
                        nc.vector.tensor_copy(ots[p][0:64, qs2], po0[0:64, :])
                        nc.vector.tensor_copy(ots[p][64:128, qs2], po1[0:64, :])
                    nc.vector.tensor_copy(zr[0:1, :], po0[64:65, :])
                    nc.vector.tensor_copy(zr[32:33, :], po1[64:65, :])
                    with nc.allow_low_precision(reason="1/Z to f32r matmul"):
                        nc.vector.reciprocal(rz[0:33, :], zr[0:33, :])
                    if last:
                        nc.vector.tensor_copy(ots[p][0:64, qs2], po0[0:64, :])
                        nc.vector.tensor_copy(ots[p][64:128, qs2], po1[0:64, :])
                    if nxt is not None:
                        qk_group(p + 1, qb, *nxt)
                    pending = make_tail(p, qs2, rz)
                cur = nxt
            pending()
            oproj_group(3)

    split_multi_waits(nc)
    return nc


_CACHED_NC = None


def get_program():
    global _CACHED_NC
    if _CACHED_NC is None:
        _CACHED_NC = build_program()
    return _CACHED_NC


def _pack_w(WT, dc_out):
    """[1024 or 512 rows, cols] -> [128, rows/128 * cols] with
    [p, cols*d + c] = WT[128*d + p, c]."""
    rows, cols = WT.shape
    nd = rows // 128
    return np.ascontiguousarray(
        WT.reshape(nd, 128, cols).transpose(1, 0, 2).reshape(128, nd * cols))


def make_in_maps(x, Wq, bq, Wk, bk, Wv, bv, Wo, bo):
    x = np.asarray(x, np.float32)
    bf = ml_dtypes.bfloat16
    WqT = np.ascontiguousarray(np.asarray(Wq, np.float32).T)
    WkT = np.ascontiguousarray(np.asarray(Wk, np.float32).T)
    WvT = np.ascontiguousarray(np.asarray(Wv, np.float32).T)
    WoT = np.ascontiguousarray(np.asarray(Wo, np.float32).T)
    bq = np.asarray(bq, np.float32)
    bk = np.asarray(bk, np.float32)
    bv = np.asarray(bv, np.float32)
    shard = []
    for hg in range(2):
        ds = slice(DC * hg, DC * (hg + 1))
        shard.append({
            "wqP": _pack_w(WqT[:, ds], DC).astype(bf),
            "wkP": _pack_w(WkT[:, ds], DC).astype(bf),
            "wvP": _pack_w(WvT[:, ds], DC).astype(bf),
            "woP": _pack_w(WoT[ds, :], D).astype(bf),
            "bq2": np.ascontiguousarray(bq[ds].reshape(NP, 128).T),
            "bk2": np.ascontiguousarray(bk[ds].reshape(NP, 128).T),
            "bvb": np.ascontiguousarray(np.tile(bv[ds], (128, 1))),
            "ones4": np.ones((4, 64), np.float32),
            "tag": np.zeros((1, VARIANT), np.float32),
        })
    in_maps = []
    for c in range(NCORES):
        b, hg = c // 2, c % 2
        m = dict(shard[hg])
        # xP[p, 4096*tb + 512*d + c] = x[b][512*tb + c, 128*d + p]
        xb = x[b].reshape(4, 512, 8, 128)          # [tb, c, d, p]
        m["xP"] = np.ascontiguousarray(
            xb.transpose(3, 0, 2, 1).reshape(128, 16384)).astype(bf)
        in_maps.append(m)
    return in_maps


def assemble(results, bo):
    """Unshard: sum the two row-parallel bf16 partials per batch element
    (upcast to f32) + bias."""
    bo = np.asarray(bo, np.float32)
    out = np.empty((4, S, D), np.float32)
    for b in range(4):
        out[b] = (results[2 * b]["out"].astype(np.float32)
                  + results[2 * b + 1]["out"].astype(np.float32) + bo)
    return out


def kernel(x, Wq, bq, Wk, bk, Wv, bv, Wo, bo):
    nc = get_program()
    in_maps = make_in_maps(x, Wq, bq, Wk, bk, Wv, bv, Wo, bo)
    res = run_bass_kernel_spmd(nc, in_maps, list(range(NCORES)))
    return assemble(res.results, bo)


# revision 45
# speedup vs baseline: 1.0016x; 1.0016x over previous
"""Multi-head attention (dense transformer block) on 8 Trainium2 NeuronCores.

Sharding: (batch=4) x (head-group=2) -> 8 shards, tensor-parallel over heads.
Core c handles batch b = c//2 and heads [8*hg, 8*hg+8) with hg = c%2:
Q/K/V weights column-sharded (512 of 1024 output dims per core), Wo
row-sharded; the two row-parallel partial outputs per batch element are
summed host-side (plus the bo bias) during the unshard. No collectives.

Inputs are host-packed so every DMA moves >=8KB-contiguous per-partition
lines (descriptor-dispatch efficiency), issued in first-use order so the
first projection matmul only waits on ~2MB, not the whole 9MB input set.

Per core: V projection into a per-head 65-column interleave (trailing ones
column fuses the softmax denominator into the attn.V matmul); per head pair
p (0..3): Q/K projected transposed [128, 2048]; per 512-query block, scores
for both heads land in one 2-bank PSUM tile via a row-paired matmul pair,
one wide exp(St/8) on ACT, and one M=65 matmul per head accumulates
[V|ones].T @ Pt over the 16 key chunks.  Numerators are evacuated
unnormalized (bf16) on DVE, the Z-row reciprocal runs eagerly on DVE, and
the 1/Z broadcast (ones outer-product matmuls) + in-place normalization is
deferred one query block so the PE never stalls on it.  The O projection
is interleaved into pair 3's attention (one 512-token group per query
block), so the output drains while pair 3's ACT-bound exp stream runs.
Output is stored bf16; the host upcasts, sums the two row-parallel
partials and adds bo.

Matmul operands are bf16 (fp32 PSUM accumulation).
"""

import numpy as np
import ml_dtypes

import concourse.bass as bass
import concourse.tile as tile
import concourse.mybir as mybir
from concourse.bass_utils import run_bass_kernel_spmd

F32 = mybir.dt.float32
F32R = mybir.dt.float32r
BF16 = mybir.dt.bfloat16
EXP = mybir.ActivationFunctionType.Exp

D = 1024          # d_model
S = 2048          # sequence length (full batch element per core)
NH = 16           # heads total
NHC = 8           # heads per core
NP = 4            # head pairs per core
DH = 64           # head dim
DC = 512          # output dims per core (NHC * DH)
NCORES = 8
VARIANT = 7741     # bump to bust the HLO-signature-keyed NEFF cache


def split_multi_waits(nc):
    """The walrus build in this container accepts at most one sync-wait per
    instruction; move extra waits onto same-engine nops inserted before the
    offending instruction."""
    k = 0
    for f in nc.m.functions:
        for bb in f.blocks:
            out, changed = [], False
            for inst in bb.instructions:
                si = inst.sync_info
                waits = list(si.on_wait) if si and si.on_wait else []
                if len(waits) > 1:
                    changed = True
                    for w in waits[:-1]:
                        nop = mybir.InstNoOp(name=f"wsplit-{k}", ins=[], outs=[])
                        k += 1
                        nop.engine = inst.engine
                        nop.sync_info = mybir.SyncInfo(on_wait=[w], on_update=[])
                        nc.register_instruction(nop, overwrite=True)
                        out.append(nop)
                    si.on_wait = waits[-1:]
                out.append(inst)
            if changed:
                bb.instructions = out


def build_program():
    nc = bass.Bass()
    # packed: [p, 512*d + c] = W?T[128*d + p, c]
    wkP = nc.declare_dram_parameter("wkP", [128, 4096], BF16, isOutput=False)
    wqP = nc.declare_dram_parameter("wqP", [128, 4096], BF16, isOutput=False)
    wvP = nc.declare_dram_parameter("wvP", [128, 4096], BF16, isOutput=False)
    # packed: [p, 4096*tb + 512*d + c] = x[b][512*tb + c, 128*d + p]
    xP = nc.declare_dram_parameter("xP", [128, 16384], BF16, isOutput=False)
    # packed: [p, 1024*d + c] = WoT_shard[128*d + p, c]
    woP = nc.declare_dram_parameter("woP", [128, 4096], BF16, isOutput=False)
    bk2 = nc.declare_dram_parameter("bk2", [128, NP], F32, isOutput=False)
    bq2 = nc.declare_dram_parameter("bq2", [128, NP], F32, isOutput=False)
    bvb = nc.declare_dram_parameter("bvb", [128, DC], F32, isOutput=False)
    ones4 = nc.declare_dram_parameter("ones4", [4, 64], F32R, isOutput=False)
    tag = nc.declare_dram_parameter("tag", [1, VARIANT], F32, isOutput=False)
    out = nc.declare_dram_parameter("out", [S, D], BF16, isOutput=True)

    with tile.TileContext(nc) as tc:
        with tc.tile_pool(name="pp", bufs=1) as pp, \
             tc.tile_pool(name="qk", bufs=2) as qkp, \
             tc.tile_pool(name="pt", bufs=3) as ptp, \
             tc.tile_pool(name="zr", bufs=2) as zrp, \
             tc.tile_pool(name="osb", bufs=3) as op_, \
             tc.tile_pool(name="psS", bufs=2, space="PSUM") as stp, \
             tc.tile_pool(name="psA", bufs=4, space="PSUM") as psa:
            bk_sb = pp.tile([128, 16], F32, name="bk_sb", tag="bk_sb")
            bq_sb = pp.tile([128, 16], F32, name="bq_sb", tag="bq_sb")
            bvb_sb = pp.tile([128, DC], F32, name="bvb_sb", tag="bvb_sb")
            ones_sb = pp.tile([128, 64], F32R, name="ones_sb", tag="ones_sb")
            tag_sb = pp.tile([1, VARIANT], F32, name="tag_sb", tag="tag_sb")
            wk_sb = pp.tile([128, 4096], BF16, name="wk_sb", tag="wk_sb")
            wq_sb = pp.tile([128, 4096], BF16, name="wq_sb", tag="wq_sb")
            wv_sb = pp.tile([128, 4096], BF16, name="wv_sb", tag="wv_sb")
            wo_sb = pp.tile([128, 4096], BF16, name="wo_sb", tag="wo_sb")
            xt_sb = pp.tile([128, 16384], BF16, name="xt_sb", tag="xt_sb")

            def wk(d, p4=None):
                a = wk_sb[:, 512 * d:512 * (d + 1)]
                return a if p4 is None else a[:, 128 * p4:128 * (p4 + 1)]

            def wq(d, p4=None):
                a = wq_sb[:, 512 * d:512 * (d + 1)]
                return a if p4 is None else a[:, 128 * p4:128 * (p4 + 1)]

            def wv(d):
                return wv_sb[:, 512 * d:512 * (d + 1)]

            def wo(d, hf):
                return wo_sb[:, 1024 * d + 512 * hf:1024 * d + 512 * (hf + 1)]

            def xt512(d, tb):
                o = 4096 * tb + 512 * d
                return xt_sb[:, o:o + 512]

            def xt128(d, k):
                o = 4096 * (k // 4) + 512 * d + 128 * (k % 4)
                return xt_sb[:, o:o + 128]

            # DMAs in first-use order, split across the two HWDGE queues
            # (SP + Act) so the first projection's inputs land in parallel.
            nc.sync.dma_start(wk_sb[:, 0:2048], wkP[:, 0:2048])
            nc.scalar.dma_start(wk_sb[:, 2048:4096], wkP[:, 2048:4096])
            nc.scalar.dma_start(bk_sb[:, 0:NP], bk2[:])
            nc.scalar.dma_start(bq_sb[:, 0:NP], bq2[:])
            nc.scalar.dma_start(tag_sb[:], tag[:])
            for s in range(4):
                nc.scalar.dma_start(ones_sb[32 * s:32 * s + 1, :],
                                    ones4[s:s + 1, :])
            nc.sync.dma_start(xt_sb[:, 0:2048], xP[:, 0:2048])
            nc.scalar.dma_start(xt_sb[:, 2048:4096], xP[:, 2048:4096])
            nc.sync.dma_start(wq_sb[:], wqP[:])
            nc.scalar.dma_start(bvb_sb[:], bvb[:])
            nc.scalar.dma_start(wv_sb[:], wvP[:])
            for tb in range(1, 4):
                nc.sync.dma_start(xt_sb[:, 4096 * tb:4096 * (tb + 1)],
                                  xP[:, 4096 * tb:4096 * (tb + 1)])
            nc.scalar.dma_start(wo_sb[:], woP[:])

            vg = [pp.tile([128, 520], BF16, name=f"vg{t}", tag=f"vg{t}")
                  for t in range(16)]
            ots = [pp.tile([128, S], BF16, name=f"ot{i}", tag=f"ot{i}")
                   for i in range(NP)]

            def qk_tiles():
                kt_p = qkp.tile([128, S], BF16, name="kt_p", tag="kt", bufs=2)
                qt_p = qkp.tile([128, S], BF16, name="qt_p", tag="qt", bufs=2)
                return kt_p, qt_p

            def qk_group(p, tb, kt_p, qt_p):
                """K+Q projection for one 512-token block of head pair p."""
                ts = slice(512 * tb, 512 * (tb + 1))
                ps = psa.tile([128, 512], F32, name="psk", tag="psa", bufs=4)
                for d in range(8):
                    nc.tensor.matmul(ps[:], wk(d, p), xt512(d, tb),
                                     start=(d == 0), stop=(d == 7))
                nc.vector.tensor_scalar_add(kt_p[:, ts], ps[:],
                                            bk_sb[:, p:p + 1])
                ps = psa.tile([128, 512], F32, name="psq", tag="psa", bufs=4)
                for d in range(8):
                    nc.tensor.matmul(ps[:], wq(d, p), xt512(d, tb),
                                     start=(d == 0), stop=(d == 7))
                nc.vector.tensor_scalar_add(qt_p[:, ts], ps[:],
                                            bq_sb[:, p:p + 1])

            def make_tail(p, qs2, rz):
                """1/Z broadcast + normalization for one finished query
                block.  Emitted one block LATE so the PE's in-order stream
                never stalls (the DVE reciprocal ran a full block earlier).
                The broadcast lands in the st pool whose slots are freed
                promptly by ACT's exp, keeping the psa ring a clean 4-deep
                rotation of po0/po1/psk/psq."""
                def tail():
                    pb = stp.tile([128, 1024], F32, name="pb", tag="st",
                                  bufs=2)
                    nc.tensor.matmul(pb[0:64, 0:512], ones_sb[0:1, :],
                                     rz[0:1, :], start=True, stop=True)
                    nc.tensor.matmul(pb[0:64, 512:1024], ones_sb[32:33, :],
                                     rz[32:33, :], start=True, stop=True)
                    nc.vector.tensor_mul(ots[p][0:64, qs2], ots[p][0:64, qs2],
                                         pb[0:64, 0:512])
                    nc.vector.tensor_mul(ots[p][64:128, qs2],
                                         ots[p][64:128, qs2],
                                         pb[0:64, 512:1024])
                return tail

            def oproj_t8(t8):
                """O projection + store for one 128-token chunk."""
                osb = op_.tile([128, 1024], BF16, name="osb", tag="osb",
                               bufs=3)
                for hf in range(2):
                    ps = psa.tile([128, 512], F32, name="pso", tag="psa",
                                  bufs=4)
                    for p in range(4):
                        nc.tensor.matmul(
                            ps[:], ots[p][:, 128 * t8:128 * (t8 + 1)],
                            wo(p, hf), start=(p == 0), stop=(p == 3))
                    nc.vector.tensor_copy(osb[:, 512 * hf:512 * (hf + 1)],
                                          ps[:])
                nc.sync.dma_start(out[128 * t8:128 * (t8 + 1), :], osb[:])

            def oproj_group(g):
                """O projection + store for tokens [512g, 512(g+1))."""
                for t8 in range(4 * g, 4 * g + 4):
                    oproj_t8(t8)

            pending = None
            cur = qk_tiles()
            for tb in range(4):
                qk_group(0, tb, *cur)
            for p in range(NP):
                kt_p, qt_p = cur
                # next pair's projection groups are woven in one per query
                # block below, filling the PE bubble at each block boundary
                nxt = qk_tiles() if p + 1 < NP else None
                c0, c1 = 130 * p, 130 * p + 65
                for qb in range(4):
                    qs2 = slice(512 * qb, 512 * (qb + 1))
                    po0 = psa.tile([128, 512], F32, name="po0", tag="psa", bufs=4)
                    po1 = psa.tile([128, 512], F32, name="po1", tag="psa", bufs=4)
                    for k in range(16):
                        if k == 8 and pending is not None:
                            # previous block's 1/Z broadcast + normalization,
                            # woven mid-loop: DVE is idle here, so the mults
                            # finish long before anything reuses pb's slot
                            # or (pair 3) the O projection reads ots
                            pending()
                            pending = None
                        if p == NP - 1 and qb >= 1 and 11 <= k <= 14:
                            # pair 3's k-loop is ACT-paced (exp 1081ns vs PE
                            # 861ns per iter): weave the previous block's O
                            # projection chains in so PE stays busy instead
                            # of micro-stalling on exp, and the block ends
                            # as soon as the accumulators close
                            oproj_t8(4 * (qb - 1) + (k - 11))
                        if p == 0 and qb == 0:
                            # fused V projection for key chunk k
                            ps = psa.tile([128, 512], F32, name="psv",
                                          tag="psa", bufs=4)
                            for d in range(8):
                                nc.tensor.matmul(
                                    ps[:], xt128(d, k), wv(d),
                                    start=(d == 0), stop=(d == 7))
                            dst = vg[k][:].rearrange(
                                "p (h w) -> p h w", w=65)[:, :, 0:64]
                            nc.vector.tensor_add(
                                dst, ps[:].rearrange("p (h w) -> p h w", w=64),
                                bvb_sb[:].rearrange("p (h w) -> p h w", w=64))
                            nc.vector.memset(
                                vg[k][:].rearrange(
                                    "p (h w) -> p h w", w=65)[:, :, 64:65], 1.0)
                        ks = slice(128 * k, 128 * (k + 1))
                        st = stp.tile([128, 1024], F32, name="st", tag="st",
                                      bufs=2)
                        nc.tensor.matmul(st[:, 0:512], kt_p[0:64, ks],
                                         qt_p[0:64, qs2], start=True, stop=True)
                        nc.tensor.matmul(st[:, 512:1024], kt_p[64:128, ks],
                                         qt_p[64:128, qs2], start=True, stop=True)
                        pt = ptp.tile([128, 1024], BF16, name="pt", tag="pt",
                                      bufs=4)
                        nc.scalar.activation(pt[:], st[:], EXP, scale=0.125)
                        first, last = (k == 0), (k == 15)
                        nc.tensor.matmul(po0[0:65, :], vg[k][:, c0:c0 + 65],
                                         pt[:, 0:512], start=first, stop=last)
                        nc.tensor.matmul(po1[0:65, :], vg[k][:, c1:c1 + 65],
                                         pt[:, 512:1024], start=first, stop=last)
                    # numerator evacuation leads the DVE queue so the po
                    # slots recycle before the next block's first attnV;
                    # the Z-row copies + reciprocal follow (their consumer,
                    # the 1/Z broadcast, fires at k==8 of the next block).
                    # The very last block inverts this: nothing reuses its
                    # po slots, and the drain chain hangs on the reciprocal.
                    last = (p == NP - 1 and qb == 3)
                    zr = zrp.tile([128, 512], F32, name="zrt", tag="zrt", bufs=2)
                    rz = zrp.tile([128, 512], F32R, name="rz", tag="rz", bufs=2)
                    if not last:
                        nc.vector.tensor_copy(ots[p][0:64, qs2], po0[0:64, :])
                        nc.vector.tensor_copy(ots[p][64:128, qs2], po1[0:64, :])
                    nc.vector.tensor_copy(zr[0:1, :], po0[64:65, :])
                    nc.vector.tensor_copy(zr[32:33, :], po1[64:65, :])
                    with nc.allow_low_precision(reason="1/Z to f32r matmul"):
                        nc.vector.reciprocal(rz[0:33, :], zr[0:33, :])
                    if last:
                        nc.vector.tensor_copy(ots[p][0:64, qs2], po0[0:64, :])
                        nc.vector.tensor_copy(ots[p][64:128, qs2], po1[0:64, :])
                    if nxt is not None:
                        qk_group(p + 1, qb, *nxt)
                    pending = make_tail(p, qs2, rz)
                cur = nxt
            pending()
            oproj_group(3)

    split_multi_waits(nc)
    return nc


_CACHED_NC = None


def get_program():
    global _CACHED_NC
    if _CACHED_NC is None:
        _CACHED_NC = build_program()
    return _CACHED_NC


def _pack_w(WT, dc_out):
    """[1024 or 512 rows, cols] -> [128, rows/128 * cols] with
    [p, cols*d + c] = WT[128*d + p, c]."""
    rows, cols = WT.shape
    nd = rows // 128
    return np.ascontiguousarray(
        WT.reshape(nd, 128, cols).transpose(1, 0, 2).reshape(128, nd * cols))


def make_in_maps(x, Wq, bq, Wk, bk, Wv, bv, Wo, bo):
    x = np.asarray(x, np.float32)
    bf = ml_dtypes.bfloat16
    WqT = np.ascontiguousarray(np.asarray(Wq, np.float32).T)
    WkT = np.ascontiguousarray(np.asarray(Wk, np.float32).T)
    WvT = np.ascontiguousarray(np.asarray(Wv, np.float32).T)
    WoT = np.ascontiguousarray(np.asarray(Wo, np.float32).T)
    bq = np.asarray(bq, np.float32)
    bk = np.asarray(bk, np.float32)
    bv = np.asarray(bv, np.float32)
    shard = []
    for hg in range(2):
        ds = slice(DC * hg, DC * (hg + 1))
        shard.append({
            "wqP": _pack_w(WqT[:, ds], DC).astype(bf),
            "wkP": _pack_w(WkT[:, ds], DC).astype(bf),
            "wvP": _pack_w(WvT[:, ds], DC).astype(bf),
            "woP": _pack_w(WoT[ds, :], D).astype(bf),
            "bq2": np.ascontiguousarray(bq[ds].reshape(NP, 128).T),
            "bk2": np.ascontiguousarray(bk[ds].reshape(NP, 128).T),
            "bvb": np.ascontiguousarray(np.tile(bv[ds], (128, 1))),
            "ones4": np.ones((4, 64), np.float32),
            "tag": np.zeros((1, VARIANT), np.float32),
        })
    in_maps = []
    for c in range(NCORES):
        b, hg = c // 2, c % 2
        m = dict(shard[hg])
        # xP[p, 4096*tb + 512*d + c] = x[b][512*tb + c, 128*d + p]
        xb = x[b].reshape(4, 512, 8, 128)          # [tb, c, d, p]
        m["xP"] = np.ascontiguousarray(
            xb.transpose(3, 0, 2, 1).reshape(128, 16384)).astype(bf)
        in_maps.append(m)
    return in_maps


def assemble(results, bo):
    """Unshard: sum the two row-parallel bf16 partials per batch element
    (upcast to f32) + bias."""
    bo = np.asarray(bo, np.float32)
    out = np.empty((4, S, D), np.float32)
    for b in range(4):
        out[b] = (results[2 * b]["out"].astype(np.float32)
                  + results[2 * b + 1]["out"].astype(np.float32) + bo)
    return out


def kernel(x, Wq, bq, Wk, bk, Wv, bv, Wo, bo):
    nc = get_program()
    in_maps = make_in_maps(x, Wq, bq, Wk, bk, Wv, bv, Wo, bo)
    res = run_bass_kernel_spmd(nc, in_maps, list(range(NCORES)))
    return assemble(res.results, bo)


# revision 47
# speedup vs baseline: 1.0062x; 1.0046x over previous
"""Multi-head attention (dense transformer block) on 8 Trainium2 NeuronCores.

Sharding: (batch=4) x (head-group=2) -> 8 shards, tensor-parallel over heads.
Core c handles batch b = c//2 and heads [8*hg, 8*hg+8) with hg = c%2:
Q/K/V weights column-sharded (512 of 1024 output dims per core), Wo
row-sharded; the two row-parallel partial outputs per batch element are
summed host-side (plus the bo bias) during the unshard. No collectives.

Inputs are host-packed so every DMA moves >=8KB-contiguous per-partition
lines (descriptor-dispatch efficiency), issued in first-use order so the
first projection matmul only waits on ~2MB, not the whole 9MB input set.

Per core: V projection into a per-head 65-column interleave (trailing ones
column fuses the softmax denominator into the attn.V matmul); per head pair
p (0..3): Q/K projected transposed [128, 2048]; per 512-query block, scores
for both heads land in one 2-bank PSUM tile via a row-paired matmul pair,
one wide exp(St/8) on ACT, and one M=65 matmul per head accumulates
[V|ones].T @ Pt over the 16 key chunks.  Numerators are evacuated
unnormalized (bf16) on DVE, the Z-row reciprocal runs eagerly on DVE, and
the 1/Z broadcast (ones outer-product matmuls) + in-place normalization is
deferred one query block so the PE never stalls on it.  The O projection
is interleaved into pair 3's attention (one 512-token group per query
block), so the output drains while pair 3's ACT-bound exp stream runs.
Output is stored bf16; the host upcasts, sums the two row-parallel
partials and adds bo.

Matmul operands are bf16 (fp32 PSUM accumulation).
"""

import numpy as np
import ml_dtypes

import concourse.bass as bass
import concourse.tile as tile
import concourse.mybir as mybir
from concourse.bass_utils import run_bass_kernel_spmd

F32 = mybir.dt.float32
F32R = mybir.dt.float32r
BF16 = mybir.dt.bfloat16
EXP = mybir.ActivationFunctionType.Exp

D = 1024          # d_model
S = 2048          # sequence length (full batch element per core)
NH = 16           # heads total
NHC = 8           # heads per core
NP = 4            # head pairs per core
DH = 64           # head dim
DC = 512          # output dims per core (NHC * DH)
NCORES = 8
VARIANT = 7742     # bump to bust the HLO-signature-keyed NEFF cache


def split_multi_waits(nc):
    """The walrus build in this container accepts at most one sync-wait per
    instruction; move extra waits onto same-engine nops inserted before the
    offending instruction."""
    k = 0
    for f in nc.m.functions:
        for bb in f.blocks:
            out, changed = [], False
            for inst in bb.instructions:
                si = inst.sync_info
                waits = list(si.on_wait) if si and si.on_wait else []
                if len(waits) > 1:
                    changed = True
                    for w in waits[:-1]:
                        nop = mybir.InstNoOp(name=f"wsplit-{k}", ins=[], outs=[])
                        k += 1
                        nop.engine = inst.engine
                        nop.sync_info = mybir.SyncInfo(on_wait=[w], on_update=[])
                        nc.register_instruction(nop, overwrite=True)
                        out.append(nop)
                    si.on_wait = waits[-1:]
                out.append(inst)
            if changed:
                bb.instructions = out


def build_program():
    nc = bass.Bass()
    # packed: [p, 512*d + c] = W?T[128*d + p, c]
    wkP = nc.declare_dram_parameter("wkP", [128, 4096], BF16, isOutput=False)
    wqP = nc.declare_dram_parameter("wqP", [128, 4096], BF16, isOutput=False)
    wvP = nc.declare_dram_parameter("wvP", [128, 4096], BF16, isOutput=False)
    # packed: [p, 4096*tb + 512*d + c] = x[b][512*tb + c, 128*d + p]
    xP = nc.declare_dram_parameter("xP", [128, 16384], BF16, isOutput=False)
    # packed: [p, 1024*d + c] = WoT_shard[128*d + p, c]
    woP = nc.declare_dram_parameter("woP", [128, 4096], BF16, isOutput=False)
    bk2 = nc.declare_dram_parameter("bk2", [128, NP], F32, isOutput=False)
    bq2 = nc.declare_dram_parameter("bq2", [128, NP], F32, isOutput=False)
    bvb = nc.declare_dram_parameter("bvb", [128, DC], F32, isOutput=False)
    ones4 = nc.declare_dram_parameter("ones4", [4, 64], F32R, isOutput=False)
    tag = nc.declare_dram_parameter("tag", [1, VARIANT], F32, isOutput=False)
    out = nc.declare_dram_parameter("out", [S, D], BF16, isOutput=True)

    with tile.TileContext(nc) as tc:
        with tc.tile_pool(name="pp", bufs=1) as pp, \
             tc.tile_pool(name="qk", bufs=2) as qkp, \
             tc.tile_pool(name="pt", bufs=3) as ptp, \
             tc.tile_pool(name="zr", bufs=2) as zrp, \
             tc.tile_pool(name="osb", bufs=4) as op_, \
             tc.tile_pool(name="psS", bufs=2, space="PSUM") as stp, \
             tc.tile_pool(name="psA", bufs=4, space="PSUM") as psa:
            bk_sb = pp.tile([128, 16], F32, name="bk_sb", tag="bk_sb")
            bq_sb = pp.tile([128, 16], F32, name="bq_sb", tag="bq_sb")
            bvb_sb = pp.tile([128, DC], F32, name="bvb_sb", tag="bvb_sb")
            ones_sb = pp.tile([128, 64], F32R, name="ones_sb", tag="ones_sb")
            tag_sb = pp.tile([1, VARIANT], F32, name="tag_sb", tag="tag_sb")
            wk_sb = pp.tile([128, 4096], BF16, name="wk_sb", tag="wk_sb")
            wq_sb = pp.tile([128, 4096], BF16, name="wq_sb", tag="wq_sb")
            wv_sb = pp.tile([128, 4096], BF16, name="wv_sb", tag="wv_sb")
            wo_sb = pp.tile([128, 4096], BF16, name="wo_sb", tag="wo_sb")
            xt_sb = pp.tile([128, 16384], BF16, name="xt_sb", tag="xt_sb")

            def wk(d, p4=None):
                a = wk_sb[:, 512 * d:512 * (d + 1)]
                return a if p4 is None else a[:, 128 * p4:128 * (p4 + 1)]

            def wq(d, p4=None):
                a = wq_sb[:, 512 * d:512 * (d + 1)]
                return a if p4 is None else a[:, 128 * p4:128 * (p4 + 1)]

            def wv(d):
                return wv_sb[:, 512 * d:512 * (d + 1)]

            def wo(d, hf):
                return wo_sb[:, 1024 * d + 512 * hf:1024 * d + 512 * (hf + 1)]

            def xt512(d, tb):
                o = 4096 * tb + 512 * d
                return xt_sb[:, o:o + 512]

            def xt128(d, k):
                o = 4096 * (k // 4) + 512 * d + 128 * (k % 4)
                return xt_sb[:, o:o + 128]

            # DMAs in first-use order, split across the two HWDGE queues
            # (SP + Act) so the first projection's inputs land in parallel.
            nc.sync.dma_start(wk_sb[:, 0:2048], wkP[:, 0:2048])
            nc.scalar.dma_start(wk_sb[:, 2048:4096], wkP[:, 2048:4096])
            nc.scalar.dma_start(bk_sb[:, 0:NP], bk2[:])
            nc.scalar.dma_start(bq_sb[:, 0:NP], bq2[:])
            nc.scalar.dma_start(tag_sb[:], tag[:])
            for s in range(4):
                nc.scalar.dma_start(ones_sb[32 * s:32 * s + 1, :],
                                    ones4[s:s + 1, :])
            nc.sync.dma_start(xt_sb[:, 0:2048], xP[:, 0:2048])
            nc.scalar.dma_start(xt_sb[:, 2048:4096], xP[:, 2048:4096])
            nc.sync.dma_start(wq_sb[:], wqP[:])
            nc.scalar.dma_start(bvb_sb[:], bvb[:])
            nc.scalar.dma_start(wv_sb[:], wvP[:])
            for tb in range(1, 4):
                nc.sync.dma_start(xt_sb[:, 4096 * tb:4096 * (tb + 1)],
                                  xP[:, 4096 * tb:4096 * (tb + 1)])
            nc.scalar.dma_start(wo_sb[:], woP[:])

            vg = [pp.tile([128, 520], BF16, name=f"vg{t}", tag=f"vg{t}")
                  for t in range(16)]
            ots = [pp.tile([128, S], BF16, name=f"ot{i}", tag=f"ot{i}")
                   for i in range(NP)]

            def qk_tiles():
                kt_p = qkp.tile([128, S], BF16, name="kt_p", tag="kt", bufs=2)
                qt_p = qkp.tile([128, S], BF16, name="qt_p", tag="qt", bufs=2)
                return kt_p, qt_p

            def qk_group(p, tb, kt_p, qt_p):
                """K+Q projection for one 512-token block of head pair p."""
                ts = slice(512 * tb, 512 * (tb + 1))
                ps = psa.tile([128, 512], F32, name="psk", tag="psa", bufs=4)
                for d in range(8):
                    nc.tensor.matmul(ps[:], wk(d, p), xt512(d, tb),
                                     start=(d == 0), stop=(d == 7))
                nc.vector.tensor_scalar_add(kt_p[:, ts], ps[:],
                                            bk_sb[:, p:p + 1])
                ps = psa.tile([128, 512], F32, name="psq", tag="psa", bufs=4)
                for d in range(8):
                    nc.tensor.matmul(ps[:], wq(d, p), xt512(d, tb),
                                     start=(d == 0), stop=(d == 7))
                nc.vector.tensor_scalar_add(qt_p[:, ts], ps[:],
                                            bq_sb[:, p:p + 1])

            def make_tail(p, qs2, rz):
                """1/Z broadcast + normalization for one finished query
                block.  Emitted one block LATE so the PE's in-order stream
                never stalls (the DVE reciprocal ran a full block earlier).
                The broadcast lands in the st pool whose slots are freed
                promptly by ACT's exp, keeping the psa ring a clean 4-deep
                rotation of po0/po1/psk/psq."""
                def tail():
                    pb = stp.tile([128, 1024], F32, name="pb", tag="st",
                                  bufs=2)
                    nc.tensor.matmul(pb[0:64, 0:512], ones_sb[0:1, :],
                                     rz[0:1, :], start=True, stop=True)
                    nc.tensor.matmul(pb[0:64, 512:1024], ones_sb[32:33, :],
                                     rz[32:33, :], start=True, stop=True)
                    nc.vector.tensor_mul(ots[p][0:64, qs2], ots[p][0:64, qs2],
                                         pb[0:64, 0:512])
                    nc.vector.tensor_mul(ots[p][64:128, qs2],
                                         ots[p][64:128, qs2],
                                         pb[0:64, 512:1024])
                return tail

            def oproj_t8(t8):
                """O projection + store for one 128-token chunk."""
                osb = op_.tile([128, 1024], BF16, name="osb", tag="osb",
                               bufs=3)
                for hf in range(2):
                    ps = psa.tile([128, 512], F32, name="pso", tag="psa",
                                  bufs=4)
                    for p in range(4):
                        nc.tensor.matmul(
                            ps[:], ots[p][:, 128 * t8:128 * (t8 + 1)],
                            wo(p, hf), start=(p == 0), stop=(p == 3))
                    nc.vector.tensor_copy(osb[:, 512 * hf:512 * (hf + 1)],
                                          ps[:])
                nc.sync.dma_start(out[128 * t8:128 * (t8 + 1), :], osb[:])

            def oproj_group(g):
                """O projection + store for tokens [512g, 512(g+1))."""
                for t8 in range(4 * g, 4 * g + 4):
                    oproj_t8(t8)

            pending = None
            cur = qk_tiles()
            for tb in range(4):
                qk_group(0, tb, *cur)
            for p in range(NP):
                kt_p, qt_p = cur
                # next pair's projection groups are woven in one per query
                # block below, filling the PE bubble at each block boundary
                nxt = qk_tiles() if p + 1 < NP else None
                c0, c1 = 130 * p, 130 * p + 65
                for qb in range(4):
                    qs2 = slice(512 * qb, 512 * (qb + 1))
                    po0 = psa.tile([128, 512], F32, name="po0", tag="psa", bufs=4)
                    po1 = psa.tile([128, 512], F32, name="po1", tag="psa", bufs=4)
                    for k in range(16):
                        if k == 8 and pending is not None:
                            # previous block's 1/Z broadcast + normalization,
                            # woven mid-loop: DVE is idle here, so the mults
                            # finish long before anything reuses pb's slot
                            # or (pair 3) the O projection reads ots
                            pending()
                            pending = None
                        if p == NP - 1 and qb >= 1 and 11 <= k <= 14:
                            # pair 3's k-loop is ACT-paced (exp 1081ns vs PE
                            # 861ns per iter): weave the previous block's O
                            # projection chains in so PE stays busy instead
                            # of micro-stalling on exp, and the block ends
                            # as soon as the accumulators close
                            oproj_t8(4 * (qb - 1) + (k - 11))
                        if p == 0 and qb == 0:
                            # fused V projection for key chunk k
                            ps = psa.tile([128, 512], F32, name="psv",
                                          tag="psa", bufs=4)
                            for d in range(8):
                                nc.tensor.matmul(
                                    ps[:], xt128(d, k), wv(d),
                                    start=(d == 0), stop=(d == 7))
                            dst = vg[k][:].rearrange(
                                "p (h w) -> p h w", w=65)[:, :, 0:64]
                            nc.vector.tensor_add(
                                dst, ps[:].rearrange("p (h w) -> p h w", w=64),
                                bvb_sb[:].rearrange("p (h w) -> p h w", w=64))
                            nc.vector.memset(
                                vg[k][:].rearrange(
                                    "p (h w) -> p h w", w=65)[:, :, 64:65], 1.0)
                        ks = slice(128 * k, 128 * (k + 1))
                        st = stp.tile([128, 1024], F32, name="st", tag="st",
                                      bufs=2)
                        nc.tensor.matmul(st[:, 0:512], kt_p[0:64, ks],
                                         qt_p[0:64, qs2], start=True, stop=True)
                        nc.tensor.matmul(st[:, 512:1024], kt_p[64:128, ks],
                                         qt_p[64:128, qs2], start=True, stop=True)
                        pt = ptp.tile([128, 1024], BF16, name="pt", tag="pt",
                                      bufs=3)
                        nc.scalar.activation(pt[:], st[:], EXP, scale=0.125)
                        first, last = (k == 0), (k == 15)
                        nc.tensor.matmul(po0[0:65, :], vg[k][:, c0:c0 + 65],
                                         pt[:, 0:512], start=first, stop=last)
                        nc.tensor.matmul(po1[0:65, :], vg[k][:, c1:c1 + 65],
                                         pt[:, 512:1024], start=first, stop=last)
                    # numerator evacuation leads the DVE queue so the po
                    # slots recycle before the next block's first attnV;
                    # the Z-row copies + reciprocal follow (their consumer,
                    # the 1/Z broadcast, fires at k==8 of the next block).
                    # The very last block inverts this: nothing reuses its
                    # po slots, and the drain chain hangs on the reciprocal.
                    last = (p == NP - 1 and qb == 3)
                    zr = zrp.tile([128, 512], F32, name="zrt", tag="zrt", bufs=2)
                    rz = zrp.tile([128, 512], F32R, name="rz", tag="rz", bufs=2)
                    if not last:
                        nc.vector.tensor_copy(ots[p][0:64, qs2], po0[0:64, :])
                        nc.vector.tensor_copy(ots[p][64:128, qs2], po1[0:64, :])
                    nc.vector.tensor_copy(zr[0:1, :], po0[64:65, :])
                    nc.vector.tensor_copy(zr[32:33, :], po1[64:65, :])
                    with nc.allow_low_precision(reason="1/Z to f32r matmul"):
                        nc.vector.reciprocal(rz[0:33, :], zr[0:33, :])
                    if last:
                        nc.vector.tensor_copy(ots[p][0:64, qs2], po0[0:64, :])
                        nc.vector.tensor_copy(ots[p][64:128, qs2], po1[0:64, :])
                    if nxt is not None:
                        qk_group(p + 1, qb, *nxt)
                    pending = make_tail(p, qs2, rz)
                cur = nxt
            pending()
            oproj_group(3)

    split_multi_waits(nc)
    return nc


_CACHED_NC = None


def get_program():
    global _CACHED_NC
    if _CACHED_NC is None:
        _CACHED_NC = build_program()
    return _CACHED_NC


def _pack_w(WT, dc_out):
    """[1024 or 512 rows, cols] -> [128, rows/128 * cols] with
    [p, cols*d + c] = WT[128*d + p, c]."""
    rows, cols = WT.shape
    nd = rows // 128
    return np.ascontiguousarray(
        WT.reshape(nd, 128, cols).transpose(1, 0, 2).reshape(128, nd * cols))


def make_in_maps(x, Wq, bq, Wk, bk, Wv, bv, Wo, bo):
    x = np.asarray(x, np.float32)
    bf = ml_dtypes.bfloat16
    WqT = np.ascontiguousarray(np.asarray(Wq, np.float32).T)
    WkT = np.ascontiguousarray(np.asarray(Wk, np.float32).T)
    WvT = np.ascontiguousarray(np.asarray(Wv, np.float32).T)
    WoT = np.ascontiguousarray(np.asarray(Wo, np.float32).T)
    bq = np.asarray(bq, np.float32)
    bk = np.asarray(bk, np.float32)
    bv = np.asarray(bv, np.float32)
    shard = []
    for hg in range(2):
        ds = slice(DC * hg, DC * (hg + 1))
        shard.append({
            "wqP": _pack_w(WqT[:, ds], DC).astype(bf),
            "wkP": _pack_w(WkT[:, ds], DC).astype(bf),
            "wvP": _pack_w(WvT[:, ds], DC).astype(bf),
            "woP": _pack_w(WoT[ds, :], D).astype(bf),
            "bq2": np.ascontiguousarray(bq[ds].reshape(NP, 128).T),
            "bk2": np.ascontiguousarray(bk[ds].reshape(NP, 128).T),
            "bvb": np.ascontiguousarray(np.tile(bv[ds], (128, 1))),
            "ones4": np.ones((4, 64), np.float32),
            "tag": np.zeros((1, VARIANT), np.float32),
        })
    in_maps = []
    for c in range(NCORES):
        b, hg = c // 2, c % 2
        m = dict(shard[hg])
        # xP[p, 4096*tb + 512*d + c] = x[b][512*tb + c, 128*d + p]
        xb = x[b].reshape(4, 512, 8, 128)          # [tb, c, d, p]
        m["xP"] = np.ascontiguousarray(
            xb.transpose(3, 0, 2, 1).reshape(128, 16384)).astype(bf)
        in_maps.append(m)
    return in_maps


def assemble(results, bo):
    """Unshard: sum the two row-parallel bf16 partials per batch element
    (upcast to f32) + bias."""
    bo = np.asarray(bo, np.float32)
    out = np.empty((4, S, D), np.float32)
    for b in range(4):
        out[b] = (results[2 * b]["out"].astype(np.float32)
                  + results[2 * b + 1]["out"].astype(np.float32) + bo)
    return out


def kernel(x, Wq, bq, Wk, bk, Wv, bv, Wo, bo):
    nc = get_program()
    in_maps = make_in_maps(x, Wq, bq, Wk, bk, Wv, bv, Wo, bo)
    res = run_bass_kernel_spmd(nc, in_maps, list(range(NCORES)))
    return assemble(res.results, bo)
